# revision 2
# baseline (speedup 1.0000x reference)
"""MoE feed-forward (8 experts, top-2, D=1024, H=4096) on 8 Trainium2 cores.

v2: two-segment load-balanced expert parallelism.
  - Host computes gating (fp64) and routes tokens per expert.
  - Expert token sets are split into parts of <= a chunks (A-parts) and
    <= b chunks (B-parts), 128 tokens per chunk, with exactly 8 A-parts
    and 8 B-parts. Every core runs ONE program shape: segment A
    (T_A = a*128 tokens, weight set A) + segment B (T_B = b*128 tokens,
    weight set B). This drops per-core capacity from pad512(max_e N_e)
    to ~ceil(sum_e ceil(N_e/128)/8)*128 tokens.
  - Per segment the FFN runs exactly like v1: bf16 matmuls, hidden dim
    in 4 quarters, fp32 SBUF accumulator across quarters, one y write.
  - Weight tiles are single-buffered per segment tag: the other
    segment's compute window covers the prefetch.

Self-contained: hardcodes all shapes from the problem spec.
"""

import numpy as np
import ml_dtypes

import concourse.bass as bass
import concourse.mybir as mybir
import concourse.tile as tile
from concourse.bass_utils import run_bass_kernel_spmd

F32 = mybir.dt.float32
BF16 = mybir.dt.bfloat16
NP_BF16 = ml_dtypes.bfloat16

D_MODEL = 1024
HIDDEN = 4096
N_EXPERTS = 8
TOP_K = 2
NQ = 4                    # hidden-dim quarters
HQ = HIDDEN // NQ         # 1024 hidden units per quarter
DBLK = D_MODEL // 128     # 8
JQ = HQ // 128            # 8
CT = 512                  # max token tile (matmul moving free dim)


def _ct_tiles(C):
    """Token tiles: multiples of 128 covering C, at most CT wide."""
    assert C % 128 == 0
    tiles = []
    off = 0
    while off < C:
        w = min(CT, C - off)
        tiles.append((off, w))
        off += w
    return tiles


# ---------------------------------------------------------------------------
# Walrus workaround: this container's Tile emits instructions carrying more
# sync waits than the bundled walrus accepts ("Too many sync wait commands").
# Hoist excess waits onto EventSemaphore instructions placed immediately
# before the overloaded instruction (same engine, same block) — semantically
# identical: the engine blocks on each wait in program order.
_CAP_BY_OPCODE = {"EventSemaphore": 2}
_DEFAULT_CAP = 1
_split_counter = [0]


def split_excess_waits(nc):
    for f in nc.m.functions:
        for bb in f.blocks:
            new_insts = []
            changed = False
            for inst in bb.instructions:
                si = inst.sync_info
                waits = list(si.on_wait) if si is not None else []
                cap = _CAP_BY_OPCODE.get(inst.opcode, _DEFAULT_CAP)
                if len(waits) > cap:
                    changed = True
                    excess, keep = waits[:-cap], waits[-cap:]
                    for i in range(0, len(excess), 2):
                        _split_counter[0] += 1
                        new_insts.append(mybir.InstEventSemaphore(
                            name=f"I-waitsplit-{_split_counter[0]}",
                            engine=inst.engine,
                            sync_info=mybir.SyncInfo(
                                on_wait=excess[i:i + 2], on_update=[]),
                        ))
                    inst.sync_info = mybir.SyncInfo(
                        on_wait=keep, on_update=list(si.on_update))
                new_insts.append(inst)
            if changed:
                bb.instructions = new_insts
    return nc


# ---------------------------------------------------------------------------
def build_nc2(TA, TB, act=None, reps=1, bufs_x=5, bufs_h=2, bufs_ps1=2,
              bufs_ps2=2, bufs_y=3, unroll=1):
    """Two-segment per-core FFN program.

    Inputs (pre-arranged partition-major; s in {a, b} = segment):
      xp  [D*(TA+TB)] bf16   — per ct tile: [128p][8d][W] blocks, concat,
                               segment A tiles then segment B tiles
      w1s [NQ,128,DBLK,HQ] bf16
      w2s [NQ,128,JQ,D]    bf16
      b1s [128,NQ,JQ]      f32
    Output: y [TA+TB, D] f32 (segment A rows then segment B rows).
    """
    if act is None:
        act = mybir.ActivationFunctionType.Gelu
    C = TA + TB
    segs = [("a", 0, TA)]
    if TB:
        segs.append(("b", TA, TB))
    CBLK = C // 128
    nc = bass.Bass()
    xp = nc.dram_tensor("xp", [D_MODEL * C], BF16, kind="ExternalInput")
    wts = {}
    for s, _, T in segs:
        wts[s] = (
            nc.dram_tensor(f"w1{s}", [NQ, 128, DBLK, HQ], BF16,
                           kind="ExternalInput"),
            nc.dram_tensor(f"b1{s}", [128, NQ, JQ], F32,
                           kind="ExternalInput"),
            nc.dram_tensor(f"w2{s}", [NQ, 128, JQ, D_MODEL], BF16,
                           kind="ExternalInput"),
        )
    y = nc.dram_tensor("y", [C, D_MODEL], F32, kind="ExternalOutput")

    with tile.TileContext(nc) as tc:
        with (
            tc.tile_pool(name="wpool", bufs=1) as wpool,
            tc.tile_pool(name="bpool", bufs=2) as bpool,
            tc.tile_pool(name="xpool", bufs=bufs_x) as xpool,
            tc.tile_pool(name="hpool", bufs=bufs_h) as hpool,
            tc.tile_pool(name="apool", bufs=1) as apool,
            tc.tile_pool(name="ypool", bufs=bufs_y) as ypool,
            tc.tile_pool(name="ps1", bufs=bufs_ps1, space="PSUM") as ps1,
            tc.tile_pool(name="ps2", bufs=bufs_ps2, space="PSUM") as ps2,
        ):
            sections = [(q, sid) for q in range(NQ)
                        for sid in range(len(segs))]
            pending = {}

            def emit_w_load(q, sid):
                s = segs[sid][0]
                w1d, _, w2d = wts[s]
                w1t = wpool.tile([128, DBLK, HQ], BF16,
                                 tag=f"w1{s}", name=f"w1t{s}")
                nc.scalar.dma_start(out=w1t[:], in_=w1d.ap()[q])
                w2t = wpool.tile([128, JQ, D_MODEL], BF16,
                                 tag=f"w2{s}", name=f"w2t{s}")
                nc.scalar.dma_start(out=w2t[:], in_=w2d.ap()[q])
                return (w1t, w2t)

            def emit_xb_load():
                """Rep-resident activation tiles + biases."""
                b1ts = {}
                for s, _, T in segs:
                    b1t = bpool.tile([128, NQ, JQ], F32, tag=f"b1{s}",
                                     name=f"b1t{s}")
                    nc.scalar.dma_start(out=b1t[:], in_=wts[s][1].ap())
                    b1ts[s] = b1t
                xts = {}
                for s, base_tok, T in segs:
                    lst = []
                    for off, W in _ct_tiles(T):
                        xt = xpool.tile([128, DBLK, W], BF16, tag="xt",
                                        name=f"xt{s}{off}")
                        base = 128 * DBLK * (base_tok + off)
                        nc.sync.dma_start(
                            out=xt[:],
                            in_=xp.ap()[base:base + 128 * DBLK * W]
                            .rearrange("(p d c) -> p d c", p=128, d=DBLK))
                        lst.append(xt)
                    xts[s] = lst
                return b1ts, xts

            def whole(_=None):
                yacc = apool.tile([128, CBLK, D_MODEL], F32, tag="yacc")
                b1ts, xts = emit_xb_load()
                for idx, (q, sid) in enumerate(sections):
                    s, base_tok, T = segs[sid]
                    if (q, sid) not in pending:
                        pending[(q, sid)] = emit_w_load(q, sid)
                    w1t, w2t = pending.pop((q, sid))
                    # software-pipelined prefetch: next section's weights
                    # (wraps to next rep's first section at idx == last;
                    # bufs=1 rings make the cross-iteration handle valid)
                    nq, nsid = sections[(idx + 1) % len(sections)]
                    pending[(nq, nsid)] = emit_w_load(nq, nsid)
                    b1t = b1ts[s]

                    if True:
                        for ti, (off, W) in enumerate(_ct_tiles(T)):
                            xt = xts[s][ti]
                            hT = hpool.tile([128, JQ, W], BF16, tag="hT")
                            for j in range(JQ):
                                ps = ps1.tile([128, W], F32, tag="ps")
                                for d in range(DBLK):
                                    nc.tensor.matmul(
                                        ps[:],
                                        w1t[:, d, j * 128:(j + 1) * 128],
                                        xt[:, d, :],
                                        start=(d == 0), stop=(d == DBLK - 1))
                                nc.scalar.activation(
                                    hT[:, j, :], ps[:], act,
                                    bias=b1t[:, q, j:j + 1])

                            for cs in range(W // 128):
                                cb = (base_tok + off) // 128 + cs
                                p2 = ps2.tile([128, 2, 512], F32, tag="p2")
                                for dh in range(2):
                                    for j in range(JQ):
                                        nc.tensor.matmul(
                                            p2[:, dh, :],
                                            hT[:, j, cs * 128:(cs + 1) * 128],
                                            w2t[:, j, dh * 512:(dh + 1) * 512],
                                            start=(j == 0), stop=(j == JQ - 1))
                                ya = yacc[:, cb, :]
                                if q == 0:
                                    nc.vector.tensor_copy(ya, p2[:])
                                elif q < NQ - 1:
                                    nc.vector.tensor_add(ya, ya, p2[:])
                                else:
                                    yo = ypool.tile([128, 2, 512], F32,
                                                    tag="yo")
                                    nc.vector.tensor_add(yo[:], ya, p2[:])
                                    tok = base_tok + off + cs * 128
                                    nc.sync.dma_start(
                                        out=y.ap()[tok:tok + 128, :],
                                        in_=yo[:])

            if reps == 1:
                whole()
            elif reps == unroll:
                for _ in range(reps):
                    whole()
            else:
                assert reps % unroll == 0
                with tc.For_i(0, reps // unroll, 1):
                    for _ in range(unroll):
                        whole()
    return nc


# ---------------------------------------------------------------------------
def _gating(x2d, gate_w, gate_b):
    """fp64 host gating; returns per-expert (idx, prob) matching jax top_k
    (ties -> lower index, measure-zero for random inputs)."""
    logits = x2d.astype(np.float64) @ gate_w.astype(np.float64) \
        + gate_b.astype(np.float64)
    i1 = np.argmax(logits, axis=-1)
    n = len(logits)
    ar = np.arange(n)
    v1 = logits[ar, i1]
    l2 = logits.copy()
    l2[ar, i1] = -np.inf
    i2 = np.argmax(l2, axis=-1)
    v2 = l2[ar, i2]
    m = np.maximum(v1, v2)
    e1 = np.exp(v1 - m)
    e2 = np.exp(v2 - m)
    s = e1 + e2
    p1 = (e1 / s)
    p2 = (e2 / s)
    out = []
    for e in range(N_EXPERTS):
        m1 = i1 == e
        m2 = i2 == e
        idx = np.nonzero(m1 | m2)[0]
        prob = np.where(m1, p1, p2)[idx].astype(np.float32)
        out.append((idx, prob))
    return out


def plan_parts(loads):
    """Split expert loads (token counts) into 8 A-parts (a chunks) and 8
    B-parts (b chunks), 128-token chunks. Returns (a, b, slotsA, slotsB)
    where slots* = [(expert, chunk_off, n_chunks)] * 8; expert may be None
    for an all-padding slot."""
    chunks = [-(-n // 128) for n in loads]
    total = sum(chunks)
    k0 = -(-total // 8)
    for k in range(k0, k0 + 5):
        a, b = k - k // 2, k // 2
        # brute-force x_e (A-parts per expert)
        import itertools
        best = None
        for xs in itertools.product(range(4), repeat=len(chunks)):
            if sum(xs) != 8:
                continue
            ys = []
            ok = True
            for c, x in zip(chunks, xs):
                rem = max(0, c - x * a)
                yv = -(-rem // b) if b else (1 if rem else 0)
                if b == 0 and rem:
                    ok = False
                    break
                ys.append(yv)
            if not ok or sum(ys) > 8:
                continue
            best = (xs, ys)
            break
        if best is None:
            continue
        xs, ys = best
        slotsA, slotsB = [], []
        for e, (c, x, yv) in enumerate(zip(chunks, xs, ys)):
            off = 0
            for _ in range(x):
                n = min(a, c - off)
                slotsA.append((e, off, max(0, n)))
                off += a
            for _ in range(yv):
                n = min(b, c - off)
                slotsB.append((e, off, max(0, n)))
                off += b
        while len(slotsA) < 8:
            slotsA.append((None, 0, 0))
        while len(slotsB) < 8:
            slotsB.append((None, 0, 0))
        return a, b, slotsA, slotsB
    raise RuntimeError("plan_parts: no feasible packing")


def _pack_x(xTe, C):
    """xTe [D, C] f32 -> flat [D*C] bf16 in per-tile [128p][8d][W] blocks."""
    parts = []
    for off, W in _ct_tiles(C):
        blk = xTe[:, off:off + W].reshape(DBLK, 128, W).transpose(1, 0, 2)
        parts.append(np.ascontiguousarray(blk).reshape(-1))
    return np.concatenate(parts).astype(NP_BF16)


def _pack_w(w1, b1, w2, e):
    w1p = w1[e].reshape(DBLK, 128, NQ, HQ).transpose(2, 1, 0, 3)
    w2p = w2[e].reshape(NQ, JQ, 128, D_MODEL).transpose(0, 2, 1, 3)
    b1p = b1[e].reshape(NQ, JQ, 128).transpose(2, 0, 1)
    return (np.ascontiguousarray(w1p).astype(NP_BF16),
            np.ascontiguousarray(b1p).astype(np.float32),
            np.ascontiguousarray(w2p).astype(NP_BF16))


def make_in_maps2(x2d, routes, w1, b1, w2, plan):
    a, b, slotsA, slotsB = plan
    TA, TB = a * 128, b * 128
    wcache = {}

    def wget(e):
        if e not in wcache:
            wcache[e] = _pack_w(w1, b1, w2, e)
        return wcache[e]

    in_maps = []
    for core in range(N_EXPERTS):
        m = {}
        xs = []
        for s, T, (e, coff, nch) in (("a", TA, slotsA[core]),
                                     ("b", TB, slotsB[core])):
            if T == 0:
                continue
            xTe = np.zeros((D_MODEL, T), dtype=np.float32)
            if e is not None and nch > 0:
                idx, _ = routes[e]
                lo = coff * 128
                hi = min(len(idx), lo + nch * 128)
                if hi > lo:
                    xTe[:, :hi - lo] = x2d[idx[lo:hi]].T
            xs.append(_pack_x(xTe, T))
            we = e if e is not None else 0
            w1p, b1p, w2p = wget(we)
            m[f"w1{s}"], m[f"b1{s}"], m[f"w2{s}"] = w1p, b1p, w2p
        m["xp"] = np.concatenate(xs)
        in_maps.append(m)
    return in_maps


_NC_CACHE = {}


def kernel(x, gate_w, gate_b, w1, b1, w2, b2):
    x = np.asarray(x, dtype=np.float32)
    gate_w = np.asarray(gate_w, dtype=np.float32)
    gate_b = np.asarray(gate_b, dtype=np.float32)
    w1 = np.asarray(w1, dtype=np.float32)
    b1 = np.asarray(b1, dtype=np.float32)
    w2 = np.asarray(w2, dtype=np.float32)
    b2 = np.asarray(b2, dtype=np.float32)

    B, T, D = x.shape
    x2d = x.reshape(-1, D)
    routes = _gating(x2d, gate_w, gate_b)
    plan = plan_parts([len(i) for i, _ in routes])
    a, b, slotsA, slotsB = plan
    TA, TB = a * 128, b * 128

    key = (TA, TB)
    if key not in _NC_CACHE:
        nc = build_nc2(TA, TB)
        split_excess_waits(nc)
        _NC_CACHE[key] = nc
    nc = _NC_CACHE[key]

    in_maps = make_in_maps2(x2d, routes, w1, b1, w2, plan)
    res = run_bass_kernel_spmd(nc, in_maps, core_ids=list(range(N_EXPERTS)))

    out2d = np.zeros((B * T, D_MODEL), dtype=np.float32)
    for core in range(N_EXPERTS):
        yv = res.results[core]["y"]
        for base, (e, coff, nch) in ((0, slotsA[core]), (TA, slotsB[core])):
            if e is None or nch == 0:
                continue
            idx, prob = routes[e]
            lo = coff * 128
            hi = min(len(idx), lo + nch * 128)
            if hi <= lo:
                continue
            sl = idx[lo:hi]
            y_e = yv[base:base + (hi - lo)] + b2[e]
            out2d[sl] += prob[lo:hi, None] * y_e
    return out2d.reshape(B, T, D_MODEL)
